# revision 10
# baseline (speedup 1.0000x reference)
"""DRASI encoder (MLP -> GraphConv x2 -> mu/logvar heads) on 8 Trainium2 cores.

Sharding: nodes are split into 8 contiguous shards of 6250. Each core runs the
node-local MLP on its shard (transposed layout, weights as matmul lhsT), the
shards are AllGathered into a full [50000, 128] feature table in DRAM, and
each core processes the edges whose destination lies in its shard:

  - edges are sorted by dst and bucketed into 64-node "groups"; each
    (group, src-half) bucket is padded to whole 128-edge blocks, with the
    block count unified across cores (max) so all 8 cores share one program;
  - dma_gather fetches source rows from the table (int16 indices, so the
    table is addressed as two 25000-row halves);
  - the scatter matrices S_w[e, s] = w_e * (seg_e == s) are precomputed on
    the host (they are pure edge data, identical for both conv layers) and
    streamed in on the otherwise-idle DVE DMA queue, so the segment sum is
    just per-block PE matmuls accumulating aggT = msg.T @ S_w in PSUM;
  - PSUM accumulates a whole 512-column bank (8 groups); each finished bank
    is evicted once, immediately followed by that column range's GraphConv
    linear + relu, table publish (transpose back to natural layout), and on
    the last layer the mu/logvar head, so everything pipelines behind the
    Pool-engine gather stream.

Outputs (mu, logvar) are computed per shard and concatenated on the host.
"""
import sys
sys.path.insert(0, '/opt/trn_rl_repo')

import numpy as np
import concourse.bass as bass
import concourse.bacc as bacc
import concourse.mybir as mybir
from concourse.tile import TileContext
from concourse.masks import make_identity
from concourse import bass_utils

P = 128
N_CORES = 8
N_NODES = 50000
IN_DIM = 512
HID = 128
LAT = 32
SHARD = N_NODES // N_CORES          # 6250
HALF = N_NODES // 2                 # 25000
W = 64                              # nodes per segment group
BANK = 8                            # groups per PSUM bank (512 columns)
MAXBLK = 96                         # max 128-edge blocks per gather chunk
HCAP = 56                           # max blocks per src-half within a chunk
N_GROUPS = (SHARD + W - 1) // W     # 98
N_TILES = [512] * (SHARD // 512) + ([SHARD % 512] if SHARD % 512 else [])
F32 = mybir.dt.float32
BF16 = mybir.dt.bfloat16
I16 = mybir.dt.int16
import ml_dtypes
NP_BF16 = ml_dtypes.bfloat16


# ---------------------------------------------------------------- host prep --

def _unified_structure(per_core_edges):
    """per_core_edges: list of (src, dst_local, w) sorted by dst_local.
    Returns (chunk_meta, per-core idx arrays, per-core S_w arrays)."""
    # bucket edges per core into (group, half)
    buckets = [[[None, None] for _ in range(N_GROUPS)] for _ in range(N_CORES)]
    for c, (src, dstl, wgt) in enumerate(per_core_edges):
        grp = dstl // W
        for g in range(N_GROUPS):
            sel = grp == g
            gs, gd, gw = src[sel], dstl[sel], wgt[sel]
            hi = gs >= HALF
            for h in (0, 1):
                m = hi == bool(h)
                buckets[c][g][h] = (gs[m] - h * HALF, gd[m] - g * W, gw[m])

    # unified block count per (group, half): max over cores, >= 1 block per
    # group total so every group gets a start=True matmul zeroing its PSUM
    B = np.zeros((N_GROUPS, 2), np.int64)
    for g in range(N_GROUPS):
        for h in (0, 1):
            B[g, h] = max((buckets[c][g][h][0].shape[0] + P - 1) // P
                          for c in range(N_CORES))
        if B[g, 0] == 0 and B[g, 1] == 0:
            B[g, 0] = 1

    # pack consecutive groups into chunks of <= MAXBLK blocks, with each
    # src-half capped at HCAP (separate msgL/msgH tiles)
    chunks = []
    cur, cur_lo, cur_hi = [], 0, 0
    for g in range(N_GROUPS):
        lo, hi = int(B[g, 0]), int(B[g, 1])
        if cur and (cur_lo + lo > HCAP or cur_hi + hi > HCAP
                    or cur_lo + cur_hi + lo + hi > MAXBLK):
            chunks.append(cur)
            cur, cur_lo, cur_hi = [], 0, 0
        cur.append(g)
        cur_lo += lo
        cur_hi += hi
    if cur:
        chunks.append(cur)

    chunk_meta = []
    core_idx = [[] for _ in range(N_CORES)]
    core_sw = [[] for _ in range(N_CORES)]
    for groups in chunks:
        nblk_lo = int(sum(B[g, 0] for g in groups))
        nblk_hi = int(sum(B[g, 1] for g in groups))
        nblk = nblk_lo + nblk_hi
        runs = []
        b = 0
        for h in (0, 1):
            for g in groups:
                nb = int(B[g, h])
                if nb:
                    runs.append((g, h, b, b + nb))
                    b += nb
        chunk_meta.append(dict(nblk=nblk, nblk_lo=nblk_lo, runs=runs,
                               groups=list(groups)))

        for c in range(N_CORES):
            idx_flat = np.zeros(nblk * P, np.int16)
            sw = np.zeros((P, nblk, W), np.float32)
            for (g, h, b0, b1_) in runs:
                ids, segs, ws = buckets[c][g][h]
                n = ids.shape[0]
                jj = b0 * P + np.arange(n)
                idx_flat[jj] = ids.astype(np.int16)
                sw[jj % P, jj // P, segs.astype(np.int64)] = ws
            idx_t = np.tile(idx_flat.reshape(nblk * 8, 16).T, (8, 1))
            core_idx[c].append(idx_t)
            core_sw[c].append(sw.reshape(P, nblk * W))

    edata = [np.ascontiguousarray(np.concatenate(core_idx[c], axis=1))
             for c in range(N_CORES)]
    swdata = [np.ascontiguousarray(
        np.concatenate(core_sw[c], axis=1).astype(NP_BF16))
        for c in range(N_CORES)]
    return chunk_meta, edata, swdata


# ------------------------------------------------------------- device build --

def _build(metas, idx_cols, blk_cols):
    nc = bacc.Bacc(None, target_bir_lowering=False, num_devices=N_CORES,
                   num_swdge_queues=2)

    xT = nc.dram_tensor("xT", [IN_DIM, SHARD], BF16, kind="ExternalInput")
    w1T = nc.dram_tensor("w1T", [IN_DIM, HID], BF16, kind="ExternalInput")
    b1 = nc.dram_tensor("b1", [HID, 1], F32, kind="ExternalInput")
    w2T = nc.dram_tensor("w2T", [HID, HID], BF16, kind="ExternalInput")
    b2 = nc.dram_tensor("b2", [HID, 1], F32, kind="ExternalInput")
    conv_wT = nc.dram_tensor("conv_wT", [2, 2, HID, HID], BF16, kind="ExternalInput")
    conv_b = nc.dram_tensor("conv_b", [2, HID, 1], F32, kind="ExternalInput")
    headWT = nc.dram_tensor("headWT", [HID, 2 * LAT], BF16, kind="ExternalInput")
    head_b = nc.dram_tensor("head_b", [2 * LAT, 1], F32, kind="ExternalInput")
    edata = nc.dram_tensor("edata", [P, idx_cols], I16, kind="ExternalInput")
    swd = nc.dram_tensor("swd", [P, blk_cols * W], BF16, kind="ExternalInput")
    muv_out = nc.dram_tensor("muvT", [2 * LAT, SHARD], F32, kind="ExternalOutput")

    ag_in = [nc.dram_tensor(f"ag_in{i}", [SHARD, HID], BF16) for i in range(2)]
    tables = [nc.dram_tensor(f"h_full{i}", [N_NODES, HID], BF16,
                             addr_space="Shared") for i in range(2)]

    with TileContext(nc) as tc:
        with (
            tc.tile_pool(name="const", bufs=1) as cp,
            tc.tile_pool(name="big", bufs=1) as bigp,
            tc.tile_pool(name="work", bufs=3) as wp,
            tc.tile_pool(name="natp", bufs=3) as natp,
            tc.tile_pool(name="xp", bufs=2) as xp,
            tc.tile_pool(name="edp", bufs=2) as edp,
            tc.tile_pool(name="swp", bufs=2) as swp,
            tc.tile_pool(name="msgp", bufs=3) as msgp,
            tc.tile_pool(name="muvp", bufs=2) as muvp,
            tc.tile_pool(name="ps_lin", bufs=3, space="PSUM") as ps_lin,
            tc.tile_pool(name="ps_agg", bufs=2, space="PSUM") as ps_agg,
            tc.tile_pool(name="ps_tr", bufs=2, space="PSUM") as ps_tr,
        ):
            # ---- constants (loads spread across DMA queues) ----
            w1t_sb = [cp.tile([P, HID], BF16, tag=f"w1_{k}", name=f"w1t_{k}")
                      for k in range(4)]
            for k in range(4):
                eng = nc.sync if k % 2 == 0 else nc.scalar
                eng.dma_start(out=w1t_sb[k][:], in_=w1T[k * P:(k + 1) * P, :])
            w2t_sb = cp.tile([P, HID], BF16, tag="w2")
            nc.sync.dma_start(out=w2t_sb[:], in_=w2T[:, :])
            cw_sb = [[cp.tile([P, HID], BF16, tag=f"cw{l}{m}",
                              name=f"cw_{l}_{m}") for m in range(2)]
                     for l in range(2)]
            for l in range(2):
                for m in range(2):
                    eng = nc.sync if m == 0 else nc.scalar
                    eng.dma_start(out=cw_sb[l][m][:], in_=conv_wT[l, m, :, :])
            b1_sb = cp.tile([P, 1], F32, tag="b1")
            nc.scalar.dma_start(out=b1_sb[:], in_=b1[:, :])
            b2_sb = cp.tile([P, 1], F32, tag="b2")
            nc.scalar.dma_start(out=b2_sb[:], in_=b2[:, :])
            cb_sb = [cp.tile([P, 1], F32, tag=f"cb{l}", name=f"cb_{l}")
                     for l in range(2)]
            for l in range(2):
                nc.sync.dma_start(out=cb_sb[l][:], in_=conv_b[l, :, :])
            hw_sb = cp.tile([P, 2 * LAT], BF16, tag="hw")
            nc.scalar.dma_start(out=hw_sb[:], in_=headWT[:, :])
            hb_sb = cp.tile([2 * LAT, 1], F32, tag="hb")
            nc.scalar.dma_start(out=hb_sb[:], in_=head_b[:, :])
            ident = cp.tile([P, P], BF16, tag="ident")
            make_identity(nc, ident[:])

            hA = bigp.tile([P, SHARD], BF16, tag="hA")   # h2T, then h4T
            hB = bigp.tile([P, SHARD], BF16, tag="hB")   # h3T
            aggT = bigp.tile([P, SHARD], BF16, tag="aggT")

            def emit_publish(hT_tile, t_idx, col0, nt, dma_eng=None):
                nq = (nt + P - 1) // P
                nat = natp.tile([P, 4, P], BF16, tag="nat")
                for q in range(nq):
                    n0 = col0 + q * P
                    w_ = min(P, col0 + nt - n0)
                    tr = ps_tr.tile([P, P], BF16, space="PSUM", tag="tr")
                    nc.tensor.transpose(out=tr[:w_, :],
                                        in_=hT_tile[:, n0:n0 + w_],
                                        identity=ident[:])
                    if q % 2 == 0:
                        nc.vector.tensor_copy(out=nat[:w_, q, :],
                                              in_=tr[:w_, :])
                    else:
                        nc.scalar.activation(
                            out=nat[:w_, q, :], in_=tr[:w_, :],
                            func=mybir.ActivationFunctionType.Copy)
                eng = dma_eng if dma_eng is not None else nc.sync
                if nt == 4 * P:
                    out_ap = ag_in[t_idx][col0:col0 + nt, :].rearrange(
                        "(q p) f -> p q f", q=4)
                    eng.dma_start(out=out_ap, in_=nat[:, :, :])
                else:
                    for q in range(nq):
                        n0 = col0 + q * P
                        w_ = min(P, col0 + nt - n0)
                        eng.dma_start(out=ag_in[t_idx][n0:n0 + w_, :],
                                      in_=nat[:w_, q, :])

            def emit_allgather(t_idx):
                nc.gpsimd.collective_compute(
                    "AllGather", mybir.AluOpType.bypass,
                    replica_groups=[list(range(N_CORES))],
                    ins=[ag_in[t_idx][:, :]],
                    outs=[tables[t_idx][:, :]],
                )

            # ---- MLP (bf16 matmuls, f32 psum), publishes h2 per tile.
            # DMA work alternates between the SP and (pre-collective idle)
            # Pool queues; the h1 relu runs on DVE to unload Activation. ----
            col = 0
            for j, nt in enumerate(N_TILES):
                xt = xp.tile([P, 4, 512], BF16, tag="xt")
                ld_eng = nc.gpsimd if j % 2 == 0 else nc.sync
                pub_eng = nc.sync if j % 2 == 0 else nc.gpsimd
                if nt % P == 0:
                    in_ap = xT[:, col:col + nt].rearrange(
                        "(k p) c -> p k c", k=4)
                    ld_eng.dma_start(out=xt[:, :, :nt], in_=in_ap)
                else:
                    for k in range(4):
                        ld_eng.dma_start(
                            out=xt[:, k, :nt],
                            in_=xT[k * P:(k + 1) * P, col:col + nt])
                h1_ps = ps_lin.tile([P, 512], F32, space="PSUM", tag="lin")
                for k in range(4):
                    nc.tensor.matmul(out=h1_ps[:, :nt], lhsT=w1t_sb[k][:],
                                     rhs=xt[:, k, :nt],
                                     start=(k == 0), stop=(k == 3))
                h1_sb = wp.tile([P, 512], BF16, tag="h1")
                nc.vector.tensor_scalar(
                    out=h1_sb[:, :nt], in0=h1_ps[:, :nt], scalar1=b1_sb[:],
                    scalar2=0.0, op0=mybir.AluOpType.add,
                    op1=mybir.AluOpType.max)
                h2_ps = ps_lin.tile([P, 512], F32, space="PSUM", tag="lin")
                nc.tensor.matmul(out=h2_ps[:, :nt], lhsT=w2t_sb[:],
                                 rhs=h1_sb[:, :nt], start=True, stop=True)
                nc.scalar.activation(out=hA[:, col:col + nt], in_=h2_ps[:, :nt],
                                     func=mybir.ActivationFunctionType.Relu,
                                     bias=b2_sb[:])
                emit_publish(hA, 0, col, nt, dma_eng=pub_eng)
                col += nt

            def conv_layer(layer, hT_in, hT_out, table, pub_idx=None,
                           do_head=False):
                # group -> (psum col offset within bank); banks are 8 groups
                icol = 0
                swcol = 0
                bank_ps = [None]

                def finish_bank(bank):
                    col0 = bank * BANK * W
                    bw = min(BANK * W, SHARD - col0)
                    ps = bank_ps[0]
                    nc.scalar.activation(
                        out=aggT[:, col0:col0 + bw], in_=ps[:, :bw],
                        func=mybir.ActivationFunctionType.Copy)
                    lp = ps_lin.tile([P, 512], F32, space="PSUM", tag="lin")
                    nc.tensor.matmul(out=lp[:, :bw], lhsT=cw_sb[layer][0][:],
                                     rhs=aggT[:, col0:col0 + bw],
                                     start=True, stop=False)
                    nc.tensor.matmul(out=lp[:, :bw], lhsT=cw_sb[layer][1][:],
                                     rhs=hT_in[:, col0:col0 + bw],
                                     start=False, stop=True)
                    nc.scalar.activation(out=hT_out[:, col0:col0 + bw],
                                         in_=lp[:, :bw],
                                         func=mybir.ActivationFunctionType.Relu,
                                         bias=cb_sb[layer][:])
                    if pub_idx is not None:
                        emit_publish(hT_out, pub_idx, col0, bw)
                    if do_head:
                        hp = ps_lin.tile([P, 512], F32, space="PSUM",
                                         tag="lin", name="headps")
                        nc.tensor.matmul(out=hp[:2 * LAT, :bw], lhsT=hw_sb[:],
                                         rhs=hT_out[:, col0:col0 + bw],
                                         start=True, stop=True)
                        mv = muvp.tile([2 * LAT, 512], F32, tag="mv")
                        nc.vector.tensor_tensor(
                            out=mv[:, :bw], in0=hp[:2 * LAT, :bw],
                            in1=hb_sb[:].to_broadcast([2 * LAT, bw]),
                            op=mybir.AluOpType.add)
                        nc.sync.dma_start(out=muv_out[:, col0:col0 + bw],
                                          in_=mv[:, :bw])

                for ci, meta in enumerate(metas):
                    nblk, nblk_lo = meta["nblk"], meta["nblk_lo"]
                    ed = edp.tile([P, MAXBLK * 8], I16, tag="ed")
                    nc.sync.dma_start(out=ed[:, :nblk * 8],
                                      in_=edata[:, icol:icol + nblk * 8])
                    swt = swp.tile([P, MAXBLK, W], BF16, tag="sw")
                    sw_eng = nc.sync if ci % 2 == 0 else nc.scalar
                    sw_eng.dma_start(
                        out=swt[:, :nblk, :],
                        in_=swd[:, swcol:swcol + nblk * W])

                    msgL = msgp.tile([P, HCAP, HID], BF16, tag="msgL")
                    msgH = msgp.tile([P, HCAP, HID], BF16, tag="msgH")
                    if nblk_lo:
                        nc.gpsimd.dma_gather(
                            out_ap=msgL[:, :nblk_lo, :], in_ap=table[:HALF, :],
                            idxs_ap=ed[:, :nblk_lo * 8],
                            num_idxs=nblk_lo * P, num_idxs_reg=nblk_lo * P,
                            elem_size=HID, single_packet=False,
                            queue_num=0)
                    if nblk - nblk_lo:
                        nh = nblk - nblk_lo
                        nc.gpsimd.dma_gather(
                            out_ap=msgH[:, :nh, :], in_ap=table[HALF:, :],
                            idxs_ap=ed[:, nblk_lo * 8:nblk * 8],
                            num_idxs=nh * P, num_idxs_reg=nh * P,
                            elem_size=HID, single_packet=False,
                            queue_num=0)

                    by_group = {}
                    for (g, h, b0, b1_) in meta["runs"]:
                        by_group.setdefault(g, []).append((h, b0, b1_))
                    for g in meta["groups"]:
                        if g % BANK == 0:
                            bank_ps[0] = ps_agg.tile([P, BANK * W], F32,
                                                     space="PSUM", tag="agg",
                                                     name="aggps")
                        off = (g % BANK) * W
                        ps = bank_ps[0]
                        blocks = [(h, b) for (h, b0, b1_) in by_group[g]
                                  for b in range(b0, b1_)]
                        for i, (h, b) in enumerate(blocks):
                            mt = msgL[:, b, :] if h == 0 else \
                                 msgH[:, b - nblk_lo, :]
                            nc.tensor.matmul(out=ps[:, off:off + W], lhsT=mt,
                                             rhs=swt[:, b, :],
                                             start=(i == 0),
                                             stop=(i == len(blocks) - 1))
                        if g % BANK == BANK - 1 or g == N_GROUPS - 1:
                            finish_bank(g // BANK)
                    icol += nblk * 8
                    swcol += nblk * W

            emit_allgather(0)
            conv_layer(0, hA, hB, tables[0], pub_idx=1)
            emit_allgather(1)
            conv_layer(1, hB, hA, tables[1], do_head=True)

    nc.finalize()
    return nc


# -------------------------------------------------------------------- driver --

def _get_compiled(x, edge_index, edge_attr, weights):
    src = np.asarray(edge_index[0]).astype(np.int64)
    dst = np.asarray(edge_index[1]).astype(np.int64)
    wgt = np.asarray(edge_attr, dtype=np.float32)
    x = np.asarray(x, dtype=np.float32)

    per_core_edges = []
    for c in range(N_CORES):
        sel = (dst >= c * SHARD) & (dst < (c + 1) * SHARD)
        s, d, wv = src[sel], dst[sel] - c * SHARD, wgt[sel]
        order = np.argsort(d, kind="stable")
        per_core_edges.append((s[order], d[order], wv[order]))

    metas, edata, swdata = _unified_structure(per_core_edges)
    idx_cols = sum(m["nblk"] * 8 for m in metas)
    blk_cols = sum(m["nblk"] for m in metas)

    nc = _build(metas, idx_cols, blk_cols)

    (W1, b1, W2, b2, g1_rel_W, g1_rel_b, g1_root_W,
     g2_rel_W, g2_rel_b, g2_root_W, mu_W, mu_b, lv_W, lv_b) = [
        np.asarray(w, dtype=np.float32) for w in weights]

    conv_wT = np.stack([
        np.stack([g1_rel_W.T, g1_root_W.T]),
        np.stack([g2_rel_W.T, g2_root_W.T]),
    ]).astype(NP_BF16).copy()
    conv_b = np.stack([g1_rel_b[:, None], g2_rel_b[:, None]]).copy()
    headWT = np.ascontiguousarray(
        np.concatenate([mu_W, lv_W], axis=0).T.astype(NP_BF16))
    head_b = np.concatenate([mu_b, lv_b])[:, None].copy()

    common = dict(
        w1T=np.ascontiguousarray(W1.T.astype(NP_BF16)), b1=b1[:, None].copy(),
        w2T=np.ascontiguousarray(W2.T.astype(NP_BF16)), b2=b2[:, None].copy(),
        conv_wT=conv_wT, conv_b=conv_b, headWT=headWT, head_b=head_b,
    )
    in_maps = []
    for c in range(N_CORES):
        m = dict(common)
        m["xT"] = np.ascontiguousarray(
            x[c * SHARD:(c + 1) * SHARD, :].T.astype(NP_BF16))
        m["edata"] = edata[c]
        m["swd"] = swdata[c]
        in_maps.append(m)
    return nc, in_maps


def kernel(x, edge_index, edge_attr,
           W1, b1, W2, b2,
           g1_rel_W, g1_rel_b, g1_root_W,
           g2_rel_W, g2_rel_b, g2_root_W,
           mu_W, mu_b, lv_W, lv_b):
    weights = (W1, b1, W2, b2, g1_rel_W, g1_rel_b, g1_root_W,
               g2_rel_W, g2_rel_b, g2_root_W, mu_W, mu_b, lv_W, lv_b)
    nc, in_maps = _get_compiled(x, edge_index, edge_attr, weights)
    res = bass_utils.run_bass_kernel_spmd(nc, in_maps,
                                          core_ids=list(range(N_CORES)))
    muvT = np.concatenate([res.results[c]["muvT"] for c in range(N_CORES)],
                          axis=1)
    return (np.ascontiguousarray(muvT[:LAT, :].T),
            np.ascontiguousarray(muvT[LAT:, :].T))


# revision 14
# speedup vs baseline: 1.1403x; 1.1403x over previous
"""DRASI encoder (MLP -> GraphConv x2 -> mu/logvar heads) on 8 Trainium2 cores.

Sharding: nodes are split into 8 contiguous shards of 6250. Each core runs the
node-local MLP on its shard (transposed layout, weights as matmul lhsT), the
shards are AllGathered into a full [50000, 128] feature table in DRAM, and
each core processes the edges whose destination lies in its shard:

  - edges are sorted by dst and bucketed into 64-node "groups"; each
    (group, src-half) bucket is padded to whole 128-edge blocks, with the
    block count unified across cores (max) so all 8 cores share one program;
  - dma_gather fetches source rows from the table (int16 indices, so the
    table is addressed as two 25000-row halves);
  - the scatter matrices S_w[e, s] = w_e * (seg_e == s) are precomputed on
    the host (they are pure edge data, identical for both conv layers) and
    streamed in on the otherwise-idle DVE DMA queue, so the segment sum is
    just per-block PE matmuls accumulating aggT = msg.T @ S_w in PSUM;
  - PSUM accumulates a whole 512-column bank (8 groups); each finished bank
    is evicted once, immediately followed by that column range's GraphConv
    linear + relu, table publish (transpose back to natural layout), and on
    the last layer the mu/logvar head, so everything pipelines behind the
    Pool-engine gather stream.

Outputs (mu, logvar) are computed per shard and concatenated on the host.
"""
import sys
sys.path.insert(0, '/opt/trn_rl_repo')

import numpy as np
import concourse.bass as bass
import concourse.bacc as bacc
import concourse.mybir as mybir
from concourse.tile import TileContext
from concourse.masks import make_identity
from concourse import bass_utils

P = 128
N_CORES = 8
N_NODES = 50000
IN_DIM = 512
HID = 128
LAT = 32
SHARD = N_NODES // N_CORES          # 6250
HALF = N_NODES // 2                 # 25000
W = 64                              # nodes per segment group
BANK = 8                            # groups per PSUM bank (512 columns)
MAXBLK = 96                         # max 128-edge blocks per gather chunk
HCAP = 56                           # max blocks per src-half within a chunk
N_GROUPS = (SHARD + W - 1) // W     # 98
N_TILES = [512] * (SHARD // 512) + ([SHARD % 512] if SHARD % 512 else [])
F32 = mybir.dt.float32
BF16 = mybir.dt.bfloat16
I16 = mybir.dt.int16
import ml_dtypes
NP_BF16 = ml_dtypes.bfloat16


# ---------------------------------------------------------------- host prep --

def _unified_structure(per_core_edges):
    """per_core_edges: list of (src, dst_local, w) sorted by dst_local.
    Returns (chunk_meta, per-core idx arrays, per-core S_w arrays).

    Each core packs its edges densely (per-core slot offsets, no per-bucket
    rounding); only the per-chunk block counts and the union of (group,
    half, block) participations are unified across cores, so all 8 cores
    share one program while padding is limited to cross-core count spread."""
    # bucket edges per core into (group, half)
    buckets = [[[None, None] for _ in range(N_GROUPS)] for _ in range(N_CORES)]
    cnt = np.zeros((N_CORES, N_GROUPS, 2), np.int64)
    for c, (src, dstl, wgt) in enumerate(per_core_edges):
        grp = dstl // W
        for g in range(N_GROUPS):
            sel = grp == g
            gs, gd, gw = src[sel], dstl[sel], wgt[sel]
            hi = gs >= HALF
            for h in (0, 1):
                m = hi == bool(h)
                buckets[c][g][h] = (gs[m] - h * HALF, gd[m] - g * W, gw[m])
                cnt[c, g, h] = int(m.sum())

    # a group with no edges anywhere still needs one (zero) participation so
    # its PSUM columns get a start=True matmul: give it 1 dummy lo slot
    dummy = (cnt.sum(axis=(0, 2)) == 0)
    eff = cnt.copy()
    eff[:, dummy, 0] = 1

    # pack consecutive groups into chunks: per-core dense slot totals, block
    # counts bounded by HCAP per src-half and MAXBLK overall
    chunks = []
    cur = []
    cur_lo = np.zeros(N_CORES, np.int64)
    cur_hi = np.zeros(N_CORES, np.int64)
    for g in range(N_GROUPS):
        lo, hi = eff[:, g, 0], eff[:, g, 1]
        nb_lo = -(-int((cur_lo + lo).max()) // P)
        nb_hi = -(-int((cur_hi + hi).max()) // P)
        if cur and (nb_lo > HCAP or nb_hi > HCAP or nb_lo + nb_hi > MAXBLK):
            chunks.append(cur)
            cur = []
            cur_lo[:] = 0
            cur_hi[:] = 0
        cur.append(g)
        cur_lo += lo
        cur_hi += hi
    if cur:
        chunks.append(cur)

    chunk_meta = []
    core_idx = [[] for _ in range(N_CORES)]
    core_sw = [[] for _ in range(N_CORES)]
    for groups in chunks:
        # per-core dense offsets per half
        offs = np.zeros((N_CORES, len(groups), 2), np.int64)
        tot = np.zeros((N_CORES, 2), np.int64)
        for gi, g in enumerate(groups):
            for h in (0, 1):
                offs[:, gi, h] = tot[:, h]
                tot[:, h] += eff[:, g, h]
        nblk_lo = -(-int(tot[:, 0].max()) // P)
        nblk_hi = -(-int(tot[:, 1].max()) // P)
        nblk = nblk_lo + nblk_hi

        # union participation structure over cores
        pset = set()
        for gi, g in enumerate(groups):
            for h in (0, 1):
                for c in range(N_CORES):
                    n = int(eff[c, g, h])
                    if n:
                        o = int(offs[c, gi, h])
                        for b in range(o // P, (o + n - 1) // P + 1):
                            pset.add((h, b, g))
        parts = sorted(pset)
        pidx = {k: i for i, k in enumerate(parts)}
        gplists = {g: [] for g in groups}
        for (h, b, g) in parts:
            gplists[g].append((h, b, pidx[(h, b, g)]))
        n_part = len(parts)
        chunk_meta.append(dict(nblk=nblk, nblk_lo=nblk_lo, n_part=n_part,
                               groups=list(groups), gplists=gplists))

        for c in range(N_CORES):
            idx_flat = np.zeros(nblk * P, np.int16)
            sw = np.zeros((P, n_part, W), np.float32)
            for gi, g in enumerate(groups):
                for h in (0, 1):
                    ids, segs, ws = buckets[c][g][h]
                    n = ids.shape[0]
                    if not n:
                        continue
                    o = int(offs[c, gi, h])
                    jj = o + np.arange(n)
                    bb = jj // P
                    pp = jj % P
                    idx_flat[(h * nblk_lo * P) + jj] = ids.astype(np.int16)
                    pis = np.array([pidx[(h, int(b), g)] for b in bb])
                    sw[pp, pis, segs.astype(np.int64)] = ws
            idx_t = np.tile(idx_flat.reshape(nblk * 8, 16).T, (8, 1))
            core_idx[c].append(idx_t)
            core_sw[c].append(sw.reshape(P, n_part * W))

    edata = [np.ascontiguousarray(np.concatenate(core_idx[c], axis=1))
             for c in range(N_CORES)]
    swdata = [np.ascontiguousarray(
        np.concatenate(core_sw[c], axis=1).astype(NP_BF16))
        for c in range(N_CORES)]
    return chunk_meta, edata, swdata


# ------------------------------------------------------------- device build --

def _build(metas, idx_cols, blk_cols):
    MAXPART = max(m["n_part"] for m in metas)
    nc = bacc.Bacc(None, target_bir_lowering=False, num_devices=N_CORES,
                   num_swdge_queues=2)

    xT = nc.dram_tensor("xT", [IN_DIM, SHARD], BF16, kind="ExternalInput")
    w1T = nc.dram_tensor("w1T", [IN_DIM, HID], BF16, kind="ExternalInput")
    b1 = nc.dram_tensor("b1", [HID, 1], F32, kind="ExternalInput")
    w2T = nc.dram_tensor("w2T", [HID, HID], BF16, kind="ExternalInput")
    b2 = nc.dram_tensor("b2", [HID, 1], F32, kind="ExternalInput")
    conv_wT = nc.dram_tensor("conv_wT", [2, 2, HID, HID], BF16, kind="ExternalInput")
    conv_b = nc.dram_tensor("conv_b", [2, HID, 1], F32, kind="ExternalInput")
    headWT = nc.dram_tensor("headWT", [HID, 2 * LAT], BF16, kind="ExternalInput")
    head_b = nc.dram_tensor("head_b", [2 * LAT, 1], F32, kind="ExternalInput")
    edata = nc.dram_tensor("edata", [P, idx_cols], I16, kind="ExternalInput")
    swd = nc.dram_tensor("swd", [P, blk_cols * W], BF16, kind="ExternalInput")
    muv_out = nc.dram_tensor("muvT", [2 * LAT, SHARD], F32, kind="ExternalOutput")

    ag_in = [nc.dram_tensor(f"ag_in{i}", [SHARD, HID], BF16) for i in range(2)]
    tables = [nc.dram_tensor(f"h_full{i}", [N_NODES, HID], BF16,
                             addr_space="Shared") for i in range(2)]

    with TileContext(nc) as tc:
        with (
            tc.tile_pool(name="const", bufs=1) as cp,
            tc.tile_pool(name="big", bufs=1) as bigp,
            tc.tile_pool(name="work", bufs=3) as wp,
            tc.tile_pool(name="natp", bufs=3) as natp,
            tc.tile_pool(name="xp", bufs=2) as xp,
            tc.tile_pool(name="edp", bufs=2) as edp,
            tc.tile_pool(name="swp", bufs=2) as swp,
            tc.tile_pool(name="msgp", bufs=3) as msgp,
            tc.tile_pool(name="muvp", bufs=2) as muvp,
            tc.tile_pool(name="ps_lin", bufs=3, space="PSUM") as ps_lin,
            tc.tile_pool(name="ps_agg", bufs=2, space="PSUM") as ps_agg,
            tc.tile_pool(name="ps_tr", bufs=2, space="PSUM") as ps_tr,
        ):
            # ---- constants (loads spread across DMA queues) ----
            w1t_sb = [cp.tile([P, HID], BF16, tag=f"w1_{k}", name=f"w1t_{k}")
                      for k in range(4)]
            for k in range(4):
                eng = nc.sync if k % 2 == 0 else nc.scalar
                eng.dma_start(out=w1t_sb[k][:], in_=w1T[k * P:(k + 1) * P, :])
            w2t_sb = cp.tile([P, HID], BF16, tag="w2")
            nc.sync.dma_start(out=w2t_sb[:], in_=w2T[:, :])
            cw_sb = [[cp.tile([P, HID], BF16, tag=f"cw{l}{m}",
                              name=f"cw_{l}_{m}") for m in range(2)]
                     for l in range(2)]
            for l in range(2):
                for m in range(2):
                    eng = nc.sync if m == 0 else nc.scalar
                    eng.dma_start(out=cw_sb[l][m][:], in_=conv_wT[l, m, :, :])
            b1_sb = cp.tile([P, 1], F32, tag="b1")
            nc.scalar.dma_start(out=b1_sb[:], in_=b1[:, :])
            b2_sb = cp.tile([P, 1], F32, tag="b2")
            nc.scalar.dma_start(out=b2_sb[:], in_=b2[:, :])
            cb_sb = [cp.tile([P, 1], F32, tag=f"cb{l}", name=f"cb_{l}")
                     for l in range(2)]
            for l in range(2):
                nc.sync.dma_start(out=cb_sb[l][:], in_=conv_b[l, :, :])
            hw_sb = cp.tile([P, 2 * LAT], BF16, tag="hw")
            nc.scalar.dma_start(out=hw_sb[:], in_=headWT[:, :])
            hb_sb = cp.tile([2 * LAT, 1], F32, tag="hb")
            nc.scalar.dma_start(out=hb_sb[:], in_=head_b[:, :])
            ident = cp.tile([P, P], BF16, tag="ident")
            make_identity(nc, ident[:])

            hA = bigp.tile([P, SHARD], BF16, tag="hA")   # h2T, then h4T
            hB = bigp.tile([P, SHARD], BF16, tag="hB")   # h3T
            aggT = bigp.tile([P, SHARD], BF16, tag="aggT")

            def emit_publish(hT_tile, t_idx, col0, nt, dma_eng=None):
                nq = (nt + P - 1) // P
                nat = natp.tile([P, 4, P], BF16, tag="nat")
                for q in range(nq):
                    n0 = col0 + q * P
                    w_ = min(P, col0 + nt - n0)
                    tr = ps_tr.tile([P, P], BF16, space="PSUM", tag="tr")
                    nc.tensor.transpose(out=tr[:w_, :],
                                        in_=hT_tile[:, n0:n0 + w_],
                                        identity=ident[:])
                    if q % 2 == 0:
                        nc.vector.tensor_copy(out=nat[:w_, q, :],
                                              in_=tr[:w_, :])
                    else:
                        nc.scalar.activation(
                            out=nat[:w_, q, :], in_=tr[:w_, :],
                            func=mybir.ActivationFunctionType.Copy)
                eng = dma_eng if dma_eng is not None else nc.sync
                if nt == 4 * P:
                    out_ap = ag_in[t_idx][col0:col0 + nt, :].rearrange(
                        "(q p) f -> p q f", q=4)
                    eng.dma_start(out=out_ap, in_=nat[:, :, :])
                else:
                    for q in range(nq):
                        n0 = col0 + q * P
                        w_ = min(P, col0 + nt - n0)
                        eng.dma_start(out=ag_in[t_idx][n0:n0 + w_, :],
                                      in_=nat[:w_, q, :])

            def emit_allgather(t_idx):
                nc.gpsimd.collective_compute(
                    "AllGather", mybir.AluOpType.bypass,
                    replica_groups=[list(range(N_CORES))],
                    ins=[ag_in[t_idx][:, :]],
                    outs=[tables[t_idx][:, :]],
                )

            # ---- MLP (bf16 matmuls, f32 psum), publishes h2 per tile.
            # DMA work alternates between the SP and (pre-collective idle)
            # Pool queues; the h1 relu runs on DVE to unload Activation. ----
            col = 0
            for j, nt in enumerate(N_TILES):
                xt = xp.tile([P, 4, 512], BF16, tag="xt")
                ld_eng = nc.gpsimd if j % 2 == 0 else nc.sync
                pub_eng = nc.sync if j % 2 == 0 else nc.gpsimd
                if nt % P == 0:
                    in_ap = xT[:, col:col + nt].rearrange(
                        "(k p) c -> p k c", k=4)
                    ld_eng.dma_start(out=xt[:, :, :nt], in_=in_ap)
                else:
                    for k in range(4):
                        ld_eng.dma_start(
                            out=xt[:, k, :nt],
                            in_=xT[k * P:(k + 1) * P, col:col + nt])
                h1_ps = ps_lin.tile([P, 512], F32, space="PSUM", tag="lin")
                for k in range(4):
                    nc.tensor.matmul(out=h1_ps[:, :nt], lhsT=w1t_sb[k][:],
                                     rhs=xt[:, k, :nt],
                                     start=(k == 0), stop=(k == 3))
                h1_sb = wp.tile([P, 512], BF16, tag="h1")
                nc.vector.tensor_scalar(
                    out=h1_sb[:, :nt], in0=h1_ps[:, :nt], scalar1=b1_sb[:],
                    scalar2=0.0, op0=mybir.AluOpType.add,
                    op1=mybir.AluOpType.max)
                h2_ps = ps_lin.tile([P, 512], F32, space="PSUM", tag="lin")
                nc.tensor.matmul(out=h2_ps[:, :nt], lhsT=w2t_sb[:],
                                 rhs=h1_sb[:, :nt], start=True, stop=True)
                nc.scalar.activation(out=hA[:, col:col + nt], in_=h2_ps[:, :nt],
                                     func=mybir.ActivationFunctionType.Relu,
                                     bias=b2_sb[:])
                emit_publish(hA, 0, col, nt, dma_eng=pub_eng)
                col += nt

            def conv_layer(layer, hT_in, hT_out, table, pub_idx=None,
                           do_head=False):
                # group -> (psum col offset within bank); banks are 8 groups
                icol = 0
                swcol = 0
                bank_ps = [None]

                def finish_bank(bank):
                    col0 = bank * BANK * W
                    bw = min(BANK * W, SHARD - col0)
                    ps = bank_ps[0]
                    nc.scalar.activation(
                        out=aggT[:, col0:col0 + bw], in_=ps[:, :bw],
                        func=mybir.ActivationFunctionType.Copy)
                    lp = ps_lin.tile([P, 512], F32, space="PSUM", tag="lin")
                    nc.tensor.matmul(out=lp[:, :bw], lhsT=cw_sb[layer][0][:],
                                     rhs=aggT[:, col0:col0 + bw],
                                     start=True, stop=False)
                    nc.tensor.matmul(out=lp[:, :bw], lhsT=cw_sb[layer][1][:],
                                     rhs=hT_in[:, col0:col0 + bw],
                                     start=False, stop=True)
                    nc.scalar.activation(out=hT_out[:, col0:col0 + bw],
                                         in_=lp[:, :bw],
                                         func=mybir.ActivationFunctionType.Relu,
                                         bias=cb_sb[layer][:])
                    if pub_idx is not None:
                        emit_publish(hT_out, pub_idx, col0, bw)
                    if do_head:
                        hp = ps_lin.tile([P, 512], F32, space="PSUM",
                                         tag="lin", name="headps")
                        nc.tensor.matmul(out=hp[:2 * LAT, :bw], lhsT=hw_sb[:],
                                         rhs=hT_out[:, col0:col0 + bw],
                                         start=True, stop=True)
                        mv = muvp.tile([2 * LAT, 512], F32, tag="mv")
                        nc.vector.tensor_tensor(
                            out=mv[:, :bw], in0=hp[:2 * LAT, :bw],
                            in1=hb_sb[:].to_broadcast([2 * LAT, bw]),
                            op=mybir.AluOpType.add)
                        nc.sync.dma_start(out=muv_out[:, col0:col0 + bw],
                                          in_=mv[:, :bw])

                for ci, meta in enumerate(metas):
                    nblk, nblk_lo = meta["nblk"], meta["nblk_lo"]
                    n_part = meta["n_part"]
                    ed = edp.tile([P, MAXBLK * 8], I16, tag="ed")
                    nc.sync.dma_start(out=ed[:, :nblk * 8],
                                      in_=edata[:, icol:icol + nblk * 8])
                    swt = swp.tile([P, MAXPART, W], BF16, tag="sw")
                    sw_eng = nc.sync if ci % 2 == 0 else nc.scalar
                    sw_eng.dma_start(
                        out=swt[:, :n_part, :],
                        in_=swd[:, swcol:swcol + n_part * W])

                    msgL = msgp.tile([P, HCAP, HID], BF16, tag="msgL")
                    msgH = msgp.tile([P, HCAP, HID], BF16, tag="msgH")
                    if nblk_lo:
                        nc.gpsimd.dma_gather(
                            out_ap=msgL[:, :nblk_lo, :], in_ap=table[:HALF, :],
                            idxs_ap=ed[:, :nblk_lo * 8],
                            num_idxs=nblk_lo * P, num_idxs_reg=nblk_lo * P,
                            elem_size=HID, single_packet=False,
                            queue_num=0)
                    if nblk - nblk_lo:
                        nh = nblk - nblk_lo
                        nc.gpsimd.dma_gather(
                            out_ap=msgH[:, :nh, :], in_ap=table[HALF:, :],
                            idxs_ap=ed[:, nblk_lo * 8:nblk * 8],
                            num_idxs=nh * P, num_idxs_reg=nh * P,
                            elem_size=HID, single_packet=False,
                            queue_num=0)

                    for g in meta["groups"]:
                        if g % BANK == 0:
                            bank_ps[0] = ps_agg.tile([P, BANK * W], F32,
                                                     space="PSUM", tag="agg",
                                                     name="aggps")
                        off = (g % BANK) * W
                        ps = bank_ps[0]
                        plist = meta["gplists"][g]
                        for i, (h, b, pi) in enumerate(plist):
                            mt = msgL[:, b, :] if h == 0 else msgH[:, b, :]
                            nc.tensor.matmul(out=ps[:, off:off + W], lhsT=mt,
                                             rhs=swt[:, pi, :],
                                             start=(i == 0),
                                             stop=(i == len(plist) - 1))
                        if g % BANK == BANK - 1 or g == N_GROUPS - 1:
                            finish_bank(g // BANK)
                    icol += nblk * 8
                    swcol += n_part * W

            emit_allgather(0)
            conv_layer(0, hA, hB, tables[0], pub_idx=1)
            emit_allgather(1)
            conv_layer(1, hB, hA, tables[1], do_head=True)

    nc.finalize()
    return nc


# -------------------------------------------------------------------- driver --

def _get_compiled(x, edge_index, edge_attr, weights):
    src = np.asarray(edge_index[0]).astype(np.int64)
    dst = np.asarray(edge_index[1]).astype(np.int64)
    wgt = np.asarray(edge_attr, dtype=np.float32)
    x = np.asarray(x, dtype=np.float32)

    per_core_edges = []
    for c in range(N_CORES):
        sel = (dst >= c * SHARD) & (dst < (c + 1) * SHARD)
        s, d, wv = src[sel], dst[sel] - c * SHARD, wgt[sel]
        order = np.argsort(d, kind="stable")
        per_core_edges.append((s[order], d[order], wv[order]))

    metas, edata, swdata = _unified_structure(per_core_edges)
    idx_cols = sum(m["nblk"] * 8 for m in metas)
    blk_cols = sum(m["n_part"] for m in metas)

    nc = _build(metas, idx_cols, blk_cols)

    (W1, b1, W2, b2, g1_rel_W, g1_rel_b, g1_root_W,
     g2_rel_W, g2_rel_b, g2_root_W, mu_W, mu_b, lv_W, lv_b) = [
        np.asarray(w, dtype=np.float32) for w in weights]

    conv_wT = np.stack([
        np.stack([g1_rel_W.T, g1_root_W.T]),
        np.stack([g2_rel_W.T, g2_root_W.T]),
    ]).astype(NP_BF16).copy()
    conv_b = np.stack([g1_rel_b[:, None], g2_rel_b[:, None]]).copy()
    headWT = np.ascontiguousarray(
        np.concatenate([mu_W, lv_W], axis=0).T.astype(NP_BF16))
    head_b = np.concatenate([mu_b, lv_b])[:, None].copy()

    common = dict(
        w1T=np.ascontiguousarray(W1.T.astype(NP_BF16)), b1=b1[:, None].copy(),
        w2T=np.ascontiguousarray(W2.T.astype(NP_BF16)), b2=b2[:, None].copy(),
        conv_wT=conv_wT, conv_b=conv_b, headWT=headWT, head_b=head_b,
    )
    in_maps = []
    for c in range(N_CORES):
        m = dict(common)
        m["xT"] = np.ascontiguousarray(
            x[c * SHARD:(c + 1) * SHARD, :].T.astype(NP_BF16))
        m["edata"] = edata[c]
        m["swd"] = swdata[c]
        in_maps.append(m)
    return nc, in_maps


def kernel(x, edge_index, edge_attr,
           W1, b1, W2, b2,
           g1_rel_W, g1_rel_b, g1_root_W,
           g2_rel_W, g2_rel_b, g2_root_W,
           mu_W, mu_b, lv_W, lv_b):
    weights = (W1, b1, W2, b2, g1_rel_W, g1_rel_b, g1_root_W,
               g2_rel_W, g2_rel_b, g2_root_W, mu_W, mu_b, lv_W, lv_b)
    nc, in_maps = _get_compiled(x, edge_index, edge_attr, weights)
    res = bass_utils.run_bass_kernel_spmd(nc, in_maps,
                                          core_ids=list(range(N_CORES)))
    muvT = np.concatenate([res.results[c]["muvT"] for c in range(N_CORES)],
                          axis=1)
    return (np.ascontiguousarray(muvT[:LAT, :].T),
            np.ascontiguousarray(muvT[LAT:, :].T))
